# revision 59
# baseline (speedup 1.0000x reference)
"""Trainium2 Bass kernel for nn_BiInteraction (segment softmax bi-interaction).

Strategy (data-parallel over molecules, 8 NeuronCores):
  - Each core owns 8 molecules (its contiguous slice of the batch) and gets
    its slice of protSeq_embed in two layouts (host-transposed protT for the
    score matmuls; natural-layout chunks for the attention pool), its atoms
    padded to 64 slots per molecule (pads are REPLICAS of a real atom, which
    keeps every max reduction exact without masks), an indicator matrix for
    segment sums, and the replicated MLP weights.
  - All matmul operands are fp16 (PSUM accumulation stays fp32): single-pass
    PE matmuls (fp32 runs LOW_HIGH double-pass at 4x the cost) and half the
    HBM traffic. End-to-end error vs the fp32 reference is ~1e-3.
  - Filler matmuls keep the PE p-state ramp alive while the input DMAs are
    in flight: the PE only reaches 2.4 GHz after ~3us of gap-free execution
    and ANY idle resets it, so fillers bridge from program start to the
    first data-dependent matmul (and across the xt-cast gap).
  - Scores S[a, l] = (atom @ W_att) . prot[seg(a), l] are computed
    block-diagonally: one matmul per molecule (k = d = 128 contraction),
    two molecules stacked per PSUM bank.
  - Segment softmax over atoms:   Wc = exp(max_l S); Sc = 1^T (ind * Wc).
  - Residue softmax over protein: Wp = max_a S via PE transpose + grouped
    reduce (split vector/gpsimd); ew = exp(Wp); t = sum_l ew via ones-matmul.
    The per-stack exps fuse into a single scalar-engine activation.
  - Pools via matmuls. The protein pool is COLUMN-form (stationary = pnat
    128x128 block, moving = exp-weight column) so the pooled vector lands
    feature-on-partition directly - no transpose / copy step.
  - Normalization via a k=1 broadcast matmul + reciprocal, then the 3-layer
    MLP on-chip for all 8 molecules at once ([256]->512->256->1), ReLU+bias
    alternating scalar/vector so neither engine serializes.

All shapes are static and identical across cores (single SPMD program);
per-core differences (counts, indicators, padding) live in the DMA'd data.
"""

import numpy as np

import concourse.bacc as bacc
import concourse.bass as bass
import concourse.tile as tile
from concourse import mybir
from concourse.bass_utils import run_bass_kernel_spmd

F32 = mybir.dt.float32
F16 = mybir.dt.float16
AxX = mybir.AxisListType.X
AF = mybir.ActivationFunctionType
OP = mybir.AluOpType

A, L, D, B = 2048, 512, 128, 64
H1, H2 = 512, 256
NCORES = 8
MPC = B // NCORES            # molecules per core = 8
NPAD = 64                    # padded atom slots per molecule
NSTACK = MPC * NPAD // 128   # stacks of 128 padded atoms per core = 4

# fp16 consts tensor column layout
C_IDENT = 0        # [0, 128)   identity
C_IND = 128        # [128, 136) indicator, col = molecule
C_ONES = 136       # [136, 137) ones column
C_WO = 137         # [137, 139) Wo chunks
C_W = 139

N_FILL_PRE = 12    # fillers bridging program start -> atomw landing
N_FILL_GAP = 3     # fillers bridging XT -> xt cast -> first score

_PROGRAM_CACHE = {}


def _build_program():
    nc = bacc.Bacc("TRN2", target_bir_lowering=False, debug=False)

    # atomw = atomT | watt | consts (everything needed early, one DMA)
    AW_W = MPC * NPAD + D + C_W
    d_atomw = nc.dram_tensor("atomw", [128, AW_W], F16, kind="ExternalInput")
    CN_W = NSTACK * D
    d_cons2 = nc.dram_tensor("cons2", [128, CN_W], F16, kind="ExternalInput")
    d_protp = [
        nc.dram_tensor(f"protp{q}", [128, 2 * L], F16, kind="ExternalInput")
        for q in range(4)
    ]
    d_pnatq = [
        nc.dram_tensor(f"pnatq{q}", [128, 4 * L], F16, kind="ExternalInput")
        for q in range(2)
    ]
    d_w12 = nc.dram_tensor("w12", [128, 2 * H1 + 4 * H2], F16, kind="ExternalInput")
    d_row = nc.dram_tensor("row", [1, 129 + 4 * 128 + 2 * 128], F16, kind="ExternalInput")
    d_y = nc.dram_tensor("y", [MPC, 1], F32, kind="ExternalOutput")

    with tile.TileContext(nc) as tc:
        with (
            tc.tile_pool(name="weights", bufs=1) as wpool,
            tc.tile_pool(name="work", bufs=1) as work,
            tc.tile_pool(name="spool", bufs=4) as spool,
            tc.tile_pool(name="psum_big", bufs=3, space=bass.MemorySpace.PSUM) as pbig,
            tc.tile_pool(name="psum_q", bufs=3, space=bass.MemorySpace.PSUM) as pq,
            tc.tile_pool(name="psum_s", bufs=2, space=bass.MemorySpace.PSUM) as ps,
        ):
            # warm tile first: the PE fillers depend on it and nothing else
            warm = work.tile([128, 256], F16)
            nc.gpsimd.memset(warm[:], 0.0)

            # ---- loads: 3-queue issue, earliest-needed first -----------
            # atomw layout: [atomT-s01 | watt | consts | atomT-s23]
            AW1 = 256 + D + C_W
            atomw = wpool.tile([128, AW_W], F16)
            atomT01 = atomw[:, 0:256]
            watt = atomw[:, 256 : 256 + D]
            consts = atomw[:, 256 + D : AW1]
            atomT23 = atomw[:, AW1 : AW1 + 256]
            protp = []
            for q in range(4):
                pt = wpool.tile([128, 2 * L], F16, tag=f"protp{q}")
                protp.append(pt)
            # protp0 leads the bus: scores gate on it, while atomw's landing
            # hides behind the XT+cast chain that follows it anyway
            nc.sync.dma_start(protp[0][:], d_protp[0][:])
            nc.sync.dma_start(atomw[:], d_atomw[:])
            nc.gpsimd.dma_start(protp[1][:], d_protp[1][:])
            nc.sync.dma_start(protp[2][:], d_protp[2][:])
            nc.gpsimd.dma_start(protp[3][:], d_protp[3][:])
            protT = [protp[i // 2][:, (i % 2) * L : (i % 2 + 1) * L] for i in range(MPC)]
            # late-needed tensors ride the sync HWDGE queue behind the
            # score-critical protps; scalar keeps its SEQ free for copies;
            # gpsimd's SWDGE path reaches the bus fast (good for early data)
            pnatq = []
            for q in range(2):
                pn = wpool.tile([128, 4 * L], F16, tag=f"pnatq{q}")
                eng = nc.sync if q % 2 == 0 else nc.scalar
                eng.dma_start(pn[:], d_pnatq[q][:])
                pnatq.append(pn)
            pnat = [pnatq[i // 4][:, (i % 4) * L : (i % 4 + 1) * L] for i in range(MPC)]
            cons2 = wpool.tile([128, CN_W], F16)
            atomN = cons2[:].rearrange("p (s d) -> p s d", s=NSTACK)
            w12 = wpool.tile([128, 2 * H1 + 4 * H2], F16)
            w1 = w12[:, 0 : 2 * H1]
            w2 = w12[:, 2 * H1 :]
            row = wpool.tile([1, 129 + 4 * 128 + 2 * 128], F16)
            nc.scalar.dma_start(row[:], d_row[:])
            # queues: sync: atomw,protp0,protp2,pnatq0,cons2,w12
            #         scalar: row,pnatq1 | gpsimd: protp1,protp3

            ident = consts[:, C_IDENT : C_IDENT + 128]
            ones_col = consts[:, C_ONES : C_ONES + 1]

            # ---- PE p-state fillers: no-dep matmuls that keep the PE
            # gap-free from program start until atomw lands -------------
            ps_fill = pbig.tile([128, 256], F32, tag="big")
            for _ in range(N_FILL_PRE):
                nc.tensor.matmul(
                    ps_fill[:], warm[:, :128], warm[:], start=True, stop=True
                )

            # ---- XT = W_att.T-applied atoms: XT[d', a] -----------------
            ps_xt = pbig.tile([128, MPC * NPAD], F32, tag="big")
            nc.tensor.matmul(ps_xt[:, 0:256], watt[:], atomT01[:], start=True, stop=True)
            nc.tensor.matmul(ps_xt[:, 256:512], watt[:], atomT23[:], start=True, stop=True)
            xt = work.tile([128, MPC * NPAD], F16)
            nc.vector.tensor_copy(xt[:, 0:256], ps_xt[:, 0:256])
            nc.scalar.copy(xt[:, 256:512], ps_xt[:, 256:512])

            # fillers across the cast gap so the scores stay at 2.4 GHz
            for _ in range(N_FILL_GAP):
                nc.tensor.matmul(
                    ps_fill[:], warm[:, :128], warm[:], start=True, stop=True
                )

            # ---- scores + copies + transposes + reduces + exp ----------
            # PE emission is staggered (scores s+1 before transposes s) so
            # ready work is always within the scheduler's skip window.
            # wpe col layout per stack s: col 9s = Wc; cols 9s+1+2j+sl = Wp
            wpe = work.tile([128, 9 * NSTACK], F16)
            exw = work.tile([128, 9 * NSTACK], F16)
            wce = work.tile([128, NSTACK], F32)
            wcseg = work.tile([128, MPC], F16)
            s_psums = []
            s_sbs = []
            st_psums = []

            def emit_scores(s):
                ps_S = pbig.tile([128, L], F32, tag="big")
                s_psums.append(ps_S)
                for slot in range(2):
                    i = 2 * s + slot
                    nc.tensor.matmul(
                        ps_S[slot * NPAD : (slot + 1) * NPAD, :],
                        xt[:, i * NPAD : (i + 1) * NPAD],
                        protT[i],
                        start=True,
                        stop=True,
                    )
                s_sb = spool.tile([128, L], F16, tag="s_sb")
                s_sbs.append(s_sb)
                # Wc straight from PSUM: vector may read PSUM, so this runs
                # as soon as the scores land instead of after the copy
                nc.vector.reduce_max(
                    wpe[:, 9 * s : 9 * s + 1], ps_S[:], axis=AxX
                )
                # Pool cannot read PSUM on TRN2; one wide scalar copy is
                # cheaper than two halves (fixed access latency per op)
                nc.scalar.copy(s_sb[:], ps_S[:])

            def emit_transposes(s):
                s_sb = s_sbs[s]
                ps_st = pq.tile([128, 4 * 128], F16, tag="q")
                st_psums.append(ps_st)
                for j in range(4):
                    nc.tensor.transpose(
                        ps_st[:, j * 128 : (j + 1) * 128],
                        s_sb[:, j * 128 : (j + 1) * 128],
                        ident,
                    )
                nc.vector.reduce_max(
                    wpe[:, 9 * s + 1 : 9 * s + 9],
                    ps_st[:].rearrange("p (j g k) -> p j g k", j=4, k=NPAD),
                    axis=AxX,
                )
                # one exp for Wc+Wp; the F32 Wc copy (for tensor_scalar's
                # f32-scalar requirement) and the indicator mul ride on pool
                nc.scalar.activation(
                    exw[:, 9 * s : 9 * s + 9],
                    wpe[:, 9 * s : 9 * s + 9],
                    AF.Exp,
                )
                nc.gpsimd.tensor_copy(
                    wce[:, s : s + 1], exw[:, 9 * s : 9 * s + 1]
                )
                nc.gpsimd.tensor_scalar_mul(
                    wcseg[:, 2 * s : 2 * s + 2],
                    in0=consts[:, C_IND + 2 * s : C_IND + 2 * s + 2],
                    scalar1=wce[:, s : s + 1],
                )

            emit_scores(0)
            emit_scores(1)
            emit_transposes(0)
            emit_scores(2)
            emit_transposes(1)
            emit_scores(3)
            emit_transposes(2)
            emit_transposes(3)

            # late loads: issued after the score-critical section so their
            # bus slots trail the protps (needed only by pools / MLP)
            nc.sync.dma_start(cons2[:], d_cons2[:])
            nc.sync.dma_start(w12[:], d_w12[:])

            # ---- late-dep PE matmuls: denominators and pools -----------
            # ps_row packs the tiny partition-sum results: cols 0..8 = Sc
            # (per-stack n=2 matmuls), cols 8..40 = per-(s,j,sl) t partials.
            ps_row = ps.tile([1, 5 * MPC], F32, tag="sp")
            ps_appp = ps.tile([128, 2 * MPC], F32, tag="sp")
            ps_ap = ps_appp[:, 0:MPC]
            ps_pp = ps_appp[:, MPC : 2 * MPC]
            for s in range(NSTACK):
                nc.tensor.matmul(
                    ps_row[:, 2 * s : 2 * s + 2],
                    ones_col,
                    wcseg[:, 2 * s : 2 * s + 2],
                    start=True,
                    stop=True,
                )
                nc.tensor.matmul(
                    ps_row[:, MPC + 8 * s : MPC + 8 * s + 8],
                    ones_col,
                    exw[:, 9 * s + 1 : 9 * s + 9],
                    start=True,
                    stop=True,
                )
                nc.tensor.matmul(
                    ps_ap[:, 2 * s : 2 * s + 2],
                    atomN[:, s, :],
                    wcseg[:, 2 * s : 2 * s + 2],
                    start=True,
                    stop=True,
                )
                for slot in range(2):
                    m = 2 * s + slot
                    for j in range(4):
                        ewc = 9 * s + 1 + 2 * j + slot
                        nc.tensor.matmul(
                            ps_pp[:, m : m + 1],
                            pnat[m][:, j * 128 : (j + 1) * 128],
                            exw[:, ewc : ewc + 1],
                            start=(j == 0),
                            stop=(j == 3),
                        )

            # ---- normalizers: sct = [Sc | t], broadcast, reciprocal ----
            sct = work.tile([1, 2 * MPC], F16)
            nc.scalar.copy(sct[:, :MPC], ps_row[:, :MPC])
            with nc.allow_low_precision(reason="sum of 4 fp16 values, 5e-4 rel"):
                nc.vector.reduce_sum(
                    sct[:, MPC:].rearrange("p (s sl) -> p s sl", sl=2),
                    ps_row[:, MPC:].rearrange("p (s j sl) -> p s sl j", s=4, j=4),
                    axis=AxX,
                )
            # split normalizers: the Sc side closes early (no t dependence),
            # so htop and the h1 top-half matmuls hide inside the t wait
            inv = work.tile([128, 2 * MPC], F32)
            ps_bc_sc = pq.tile([128, MPC], F32, tag="q")
            nc.tensor.matmul(
                ps_bc_sc[:], row[:, :128], sct[:, :MPC], start=True, stop=True
            )
            nc.vector.reciprocal(inv[:, :MPC], ps_bc_sc[:])
            htop = work.tile([128, MPC], F16)
            nc.vector.tensor_mul(htop[:], ps_ap[:], inv[:, :MPC])
            ps_bc_t = ps.tile([128, MPC], F32, tag="sp")
            nc.tensor.matmul(
                ps_bc_t[:], row[:, :128], sct[:, MPC:], start=True, stop=True
            )
            nc.vector.reciprocal(inv[:, MPC:], ps_bc_t[:])
            hbot = work.tile([128, MPC], F16)
            nc.vector.tensor_mul(hbot[:], ps_pp[:], inv[:, MPC:])

            # ---- MLP: all 8 molecules at once; ReLU+bias alternates
            # scalar/vector so neither engine serializes ----------------
            # bias enters via k=1 matmuls (stationary = bias row, moving =
            # ones) so activations batch two 128-unit chunks at a time.
            h1 = work.tile([128, 4 * MPC], F16)
            h2 = work.tile([128, 2 * MPC], F16)
            for pair in range(2):
                ps_h1 = (ps if pair == 0 else pq).tile(
                    [128, 2 * MPC], F32, tag="sp" if pair == 0 else "q"
                )
                for half in range(2):
                    mc = 2 * pair + half
                    hsl = slice(half * MPC, (half + 1) * MPC)
                    nc.tensor.matmul(
                        ps_h1[:, hsl],
                        w1[:, mc * 128 : (mc + 1) * 128],
                        htop[:],
                        start=True,
                        stop=False,
                    )
                    nc.tensor.matmul(
                        ps_h1[:, hsl],
                        w1[:, H1 + mc * 128 : H1 + (mc + 1) * 128],
                        hbot[:],
                        start=False,
                        stop=False,
                    )
                    nc.tensor.matmul(
                        ps_h1[:, hsl],
                        row[:, 129 + mc * 128 : 129 + (mc + 1) * 128],
                        row[:, 0:MPC],
                        start=False,
                        stop=True,
                    )
                if pair == 0:
                    nc.scalar.activation(h1[:, 0 : 2 * MPC], ps_h1[:], AF.Relu)
                else:
                    nc.vector.tensor_scalar_max(
                        h1[:, 2 * MPC : 4 * MPC], in0=ps_h1[:], scalar1=0.0
                    )
            for mc2 in range(2):
                msl = slice(mc2 * MPC, (mc2 + 1) * MPC)
                ps_h2 = (ps if mc2 == 0 else pq).tile(
                    [128, MPC], F32, tag="sp" if mc2 == 0 else "q"
                )
                for kc in range(4):
                    nc.tensor.matmul(
                        ps_h2[:],
                        w2[:, kc * H2 + mc2 * 128 : kc * H2 + (mc2 + 1) * 128],
                        h1[:, kc * MPC : (kc + 1) * MPC],
                        start=(kc == 0),
                        stop=False,
                    )
                nc.tensor.matmul(
                    ps_h2[:],
                    row[:, 641 + mc2 * 128 : 641 + (mc2 + 1) * 128],
                    row[:, 0:MPC],
                    start=False,
                    stop=True,
                )
                if mc2 == 0:
                    nc.scalar.activation(h2[:, msl], ps_h2[:], AF.Relu)
                else:
                    nc.vector.tensor_scalar_max(
                        h2[:, msl], in0=ps_h2[:], scalar1=0.0
                    )
            ps_o = ps.tile([MPC, 1], F32, tag="sp")
            nc.tensor.matmul(
                ps_o[:], h2[:, :MPC], consts[:, C_WO : C_WO + 1], start=True, stop=False
            )
            nc.tensor.matmul(
                ps_o[:],
                h2[:, MPC : 2 * MPC],
                consts[:, C_WO + 1 : C_WO + 2],
                start=False,
                stop=False,
            )
            nc.tensor.matmul(
                ps_o[:], row[:, :MPC], row[:, 128:129], start=False, stop=True
            )
            y_sb = work.tile([MPC, 1], F32)
            nc.vector.tensor_copy(y_sb[:], ps_o[:])
            nc.sync.dma_start(d_y[:], y_sb[:])

    nc.compile()
    return nc


def _prep_inputs(atom_embed, protSeq_embed, atom_splits, W_att, W1, b1, W2, b2, Wo, bo):
    f16 = np.float16
    atom = np.asarray(atom_embed, dtype=np.float32)
    prot = np.asarray(protSeq_embed, dtype=np.float32)
    splits = np.asarray(atom_splits).astype(np.int64).ravel()
    order = np.argsort(splits, kind="stable")
    counts = np.bincount(splits, minlength=B)
    assert counts.max() <= NPAD, f"molecule with {counts.max()} atoms > NPAD={NPAD}"
    assert counts.min() >= 1, "empty molecule (reference produces NaN there)"
    offs = np.concatenate([[0], np.cumsum(counts)])

    atomP = np.empty((B, NPAD, D), np.float32)
    ind = np.zeros((B, NPAD), np.float32)
    for b in range(B):
        idx = order[offs[b] : offs[b + 1]]
        n = len(idx)
        atomP[b, :n] = atom[idx]
        atomP[b, n:] = atom[idx[0]]  # replicate a real atom: maxes stay exact
        ind[b, :n] = 1.0

    w_att = np.asarray(W_att, np.float32).astype(f16)  # [128, 128]
    w1h = (
        np.asarray(W1, np.float32)
        .reshape(2, 128, H1).transpose(1, 0, 2).reshape(128, 2 * H1).astype(f16)
    )
    w2h = (
        np.asarray(W2, np.float32)
        .reshape(4, 128, H2).transpose(1, 0, 2).reshape(128, 4 * H2).astype(f16)
    )
    woc = np.asarray(Wo, np.float32).reshape(2, 128).T.astype(f16)
    # row = [ones(128) | bo | b1 rows (4x128) | b2 rows (2x128)]
    row = np.zeros((1, 129 + 4 * 128 + 2 * 128), f16)
    row[0, :128] = 1.0
    row[0, 128] = np.asarray(bo, np.float32).ravel()[0]
    row[0, 129:641] = np.asarray(b1, np.float32).astype(f16)
    row[0, 641:897] = np.asarray(b2, np.float32).astype(f16)
    w12h = np.ascontiguousarray(np.concatenate([w1h, w2h], axis=1))

    in_maps = []
    for c in range(NCORES):
        sl = slice(c * MPC, (c + 1) * MPC)
        protT_c = np.ascontiguousarray(
            prot[sl].transpose(0, 2, 1).astype(f16)
        )  # [MPC, 128, L]
        pnat_c = np.ascontiguousarray(
            prot[sl].reshape(MPC, 4, 128, D).transpose(0, 2, 1, 3)
            .reshape(MPC, 128, L).astype(f16)
        )
        atomT_c = np.ascontiguousarray(atomP[sl].reshape(MPC * NPAD, D).T.astype(f16))
        atomN_c = np.ascontiguousarray(
            atomP[sl].reshape(NSTACK, 128, D).transpose(1, 0, 2)
            .reshape(128, NSTACK * D).astype(f16)
        )
        ind_c = np.zeros((128, MPC), f16)
        for m in range(MPC):
            s, slot = divmod(m, 2)
            ind_c[slot * NPAD : (slot + 1) * NPAD, m] = ind[c * MPC + m]
        consts = np.zeros((128, C_W), f16)
        consts[:, C_IDENT : C_IDENT + 128] = np.eye(128, dtype=f16)
        consts[:, C_IND : C_IND + MPC] = ind_c
        consts[:, C_ONES] = 1.0
        consts[:, C_WO : C_WO + 2] = woc
        im = {
            "atomw": np.ascontiguousarray(
                np.concatenate(
                    [atomT_c[:, 0:256], w_att, consts, atomT_c[:, 256:512]],
                    axis=1,
                )
            ),
            "cons2": atomN_c,
            "w12": w12h,
            "row": row,
        }
        for q in range(4):
            im[f"protp{q}"] = np.ascontiguousarray(
                protT_c[2 * q : 2 * q + 2].transpose(1, 0, 2).reshape(128, 2 * L)
            )
        for q in range(2):
            im[f"pnatq{q}"] = np.ascontiguousarray(
                pnat_c[4 * q : 4 * q + 4].transpose(1, 0, 2).reshape(128, 4 * L)
            )
        in_maps.append(im)
    return in_maps


def kernel(atom_embed, protSeq_embed, atom_splits, W_att, W1, b1, W2, b2, Wo, bo,
           _trace=False):
    if "nc" not in _PROGRAM_CACHE:
        _PROGRAM_CACHE["nc"] = _build_program()
    nc = _PROGRAM_CACHE["nc"]
    in_maps = _prep_inputs(
        atom_embed, protSeq_embed, atom_splits, W_att, W1, b1, W2, b2, Wo, bo
    )
    res = run_bass_kernel_spmd(
        nc, in_maps, core_ids=list(range(NCORES)), trace=_trace
    )
    _PROGRAM_CACHE["last_result"] = res
    out = np.concatenate([res.results[c]["y"] for c in range(NCORES)], axis=0)
    return out.astype(np.float32)
